# revision 28
# baseline (speedup 1.0000x reference)
"""Trainium2 Bass kernel for nn_Attention_1503238553757 (LSA attention).

Reference computation (per batch element):
    qkv = x @ w_qkv; q,k,v heads of dim 64
    dots = (q @ k^T) * scale[h]; diagonal masked to -inf
    attn = softmax(dots); out = attn @ v
    y = concat_heads(out) @ w_out + b_out

Sharding: data-parallel over batch (16 batches -> 2 per core x 8 cores).

Per-core plan (all matmuls fp32r = full-rate tf32-like):
  - x [1024, 512] loaded token-major, transposed on PE -> xT [512, 1024]
  - qT,kT channel-major via lhsT=w_qkv, rhs=xT    (scoresT needs ch-major)
  - v token-major via lhsT=xT, rhs=w_qkv[:, v]    (attn@V lhsT needs tok-major)
  - scoresT[j, i] = kT_h-slice @ qT_h  (keys on partitions); heads processed
    in pairs occupying PE row-groups 0-63 / 64-127 concurrently
  - expT = exp(scale_h * scoresT) via ACT (PSUM->SBUF), diag zeroed via
    affine_select (LSA self-token mask)
  - attn@V: lhsT = [v_h | ones] (M=65) accumulated over j-tiles ->
    outT[0:64] = unnormalized out^T, outT[64] = softmax denominators
  - normalize: fast reciprocal + DRAM-bounce partition-broadcast + DVE
    multiply, written as oT (inner-channel-major) = lhsT for out projection
  - y = oT.T @ w_out + b_out, token-major, DMA'd out

Emission is software-pipelined to keep the PE dense (HAM warm): attention
for batch b starts after a minimal projection prefix; batch b+1's x-load
and transposes interleave into batch b's attention pairs; batch b's out
projection interleaves into batch b+1's attention.

PSUM: psB ([128,1024] = 2 banks) x2 + psO ([65,1024] = 2 banks) x2 = 8 banks.
"""

import os
import sys

for _p in ("/opt/trn_rl_repo", "/root/.axon_site/_ro/trn_rl_repo"):
    if os.path.isdir(_p) and _p not in sys.path:
        sys.path.insert(0, _p)

import numpy as np

import concourse.bass as bass
import concourse.bacc as bacc
import concourse.tile as tile
import concourse.mybir as mybir
from concourse.bass_utils import run_bass_kernel_spmd

# Problem constants (hardcoded per harness contract)
B, N, D = 16, 1024, 512
HEADS, DH = 8, 64
N_CORES = 8
BPC = B // N_CORES  # batches per core = 2

dt = mybir.dt
F32 = dt.float32
F32R = dt.float32r
BF16 = dt.bfloat16
ATT_DT = BF16  # attention-path dtype (scores/attnV operands)
EXP = mybir.ActivationFunctionType.Exp

NT = N // 128   # token tiles = 8
VW = DH + 1     # per-head v width (v | ones)
KD = D // 128   # d/inner k-tiles = 4


def build_program():
    nc = bacc.Bacc("TRN2", target_bir_lowering=False, debug=False,
                   num_devices=N_CORES)

    x = nc.dram_tensor("x", [BPC, N, D], F32, kind="ExternalInput").ap()
    w_qkv = nc.dram_tensor("w_qkv", [D, 3 * D], F32, kind="ExternalInput").ap()
    w_out = nc.dram_tensor("w_out", [D, D], F32, kind="ExternalInput").ap()
    b_out = nc.dram_tensor("b_out", [D], F32, kind="ExternalInput").ap()
    scale = nc.dram_tensor("scale", [HEADS], F32, kind="ExternalInput").ap()
    y = nc.dram_tensor("y", [BPC, N, D], F32, kind="ExternalOutput").ap()

    ident_dram = nc.inline_tensor(np.eye(128, dtype=np.float32), name="ident")
    ones_dram = nc.inline_tensor(np.ones((128, 64), dtype=np.float32), name="ones128")

    import contextlib
    with tile.TileContext(nc) as tc, contextlib.ExitStack() as ctx:
        consts = ctx.enter_context(tc.tile_pool(name="consts", bufs=1))
        p_x = ctx.enter_context(tc.tile_pool(name="p_x", bufs=1))
        p_exp = ctx.enter_context(tc.tile_pool(name="p_exp", bufs=4))
        p_mid = ctx.enter_context(tc.tile_pool(name="p_mid", bufs=3))
        p_qk = ctx.enter_context(tc.tile_pool(name="p_qk", bufs=1))
        p_v = ctx.enter_context(tc.tile_pool(name="p_v", bufs=2))
        p_y = ctx.enter_context(tc.tile_pool(name="p_y", bufs=2))
        p_rb = ctx.enter_context(tc.tile_pool(name="p_rb", bufs=2))
        p_otmp = ctx.enter_context(tc.tile_pool(name="p_otmp", bufs=3))
        p_small = ctx.enter_context(tc.tile_pool(name="p_small", bufs=2))
        psB = ctx.enter_context(tc.tile_pool(name="psB", bufs=2, space="PSUM"))
        psO = ctx.enter_context(tc.tile_pool(name="psO", bufs=2, space="PSUM"))
        p_dram = ctx.enter_context(tc.tile_pool(name="p_dram", bufs=2, space="DRAM"))

        # ---- constants (ident first: transposes only need x + ident) ----
        ident_sb = consts.tile([128, 128], F32R)
        nc.sync.dma_start(out=ident_sb, in_=ident_dram.ap().bitcast(F32R))
        wqkv_sb = consts.tile([128, KD, 3 * D], F32R)
        nc.gpsimd.dma_start(
            out=wqkv_sb,
            in_=w_qkv.rearrange("(k p) c -> p k c", p=128).bitcast(F32R),
        )
        wout_sb = consts.tile([128, KD, D], F32R)
        nc.gpsimd.dma_start(
            out=wout_sb,
            in_=w_out.rearrange("(k p) c -> p k c", p=128).bitcast(F32R),
        )
        bout_bc = consts.tile([128, D], F32)
        nc.gpsimd.dma_start(
            out=bout_bc,
            in_=bass.AP(tensor=b_out.tensor, offset=0, ap=[[0, 128], [1, D]]),
        )
        scale_sb = consts.tile([128, HEADS], F32)
        nc.gpsimd.dma_start(
            out=scale_sb,
            in_=bass.AP(tensor=scale.tensor, offset=0, ap=[[0, 128], [1, HEADS]]),
        )

        # per-batch state kept across the pipelined emission
        xT = [None] * BPC
        qkT = [None] * BPC
        vsb = [None] * BPC
        osb = [None] * BPC

        def emit_load_x(b):
            x_sb = p_x.tile([128, NT, D], F32R, tag="x", name=f"x_sb{b}")
            src = x[b].rearrange("(r p) d -> p r d", p=128).bitcast(F32R)
            nc.sync.dma_start(out=x_sb[:, 0:NT // 2, :], in_=src[:, 0:NT // 2, :])
            nc.sync.dma_start(out=x_sb[:, NT // 2:, :], in_=src[:, NT // 2:, :])
            return x_sb

        def emit_transpose_half(b, x_sb, kd, half):
            ps_t = psB.tile([128, 512], F32R, tag="psB",
                            name=f"ps_t_{b}_{kd}_{half}")
            for rr in range(4):
                r = 4 * half + rr
                nc.tensor.transpose(
                    ps_t[:, 128 * rr:128 * rr + 128],
                    x_sb[:, r, 128 * kd:128 * kd + 128],
                    ident_sb,
                )
            nc.vector.tensor_copy(
                xT[b][:, kd, 512 * half:512 * half + 512], ps_t
            )

        def emit_transposes(b, x_sb, kds):
            for half in range(2):
                for kd in kds:
                    emit_transpose_half(b, x_sb, kd, half)

        def emit_qk_half(b, ct, nh):
            ps_qk = psB.tile([128, 512], F32, tag="psB",
                             name=f"ps_qk_{b}_{ct}_{nh}")
            for kt in range(KD):
                nc.tensor.matmul(
                    ps_qk,
                    wqkv_sb[:, kt, 128 * ct:128 * ct + 128],
                    xT[b][:, kt, 512 * nh:512 * nh + 512],
                    start=(kt == 0), stop=(kt == KD - 1),
                )
            nc.vector.tensor_copy(
                qkT[b][:, ct, 512 * nh:512 * nh + 512], ps_qk
            )

        def emit_qk_ct(b, ct):
            for nh in range(2):
                emit_qk_half(b, ct, nh)

        def emit_v_r(b, r):
            ps_v = psB.tile([128, 512], F32, tag="psB", name=f"ps_v_{b}_{r}")
            for kt in range(KD):
                nc.tensor.matmul(
                    ps_v,
                    xT[b][:, kt, 128 * r:128 * r + 128],
                    wqkv_sb[:, kt, 2 * D:3 * D],
                    start=(kt == 0), stop=(kt == KD - 1),
                )
            nc.vector.tensor_copy(
                vsb[b][:, r, 0:HEADS * VW].rearrange(
                    "p (h e) -> p h e", h=HEADS)[:, :, 0:DH],
                ps_v.rearrange("p (h e) -> p h e", h=HEADS),
            )

        def emit_ones(b):
            nc.vector.memset(
                vsb[b][:, :, 0:HEADS * VW].rearrange(
                    "p r (h e) -> p r h e", h=HEADS)[:, :, :, DH:DH + 1],
                1.0,
            )
            # pad region read as garbage weights by the last head's widened
            # attn@V lhsT; keep it finite
            nc.vector.memset(vsb[b][:, :, HEADS * VW:], 1.0)

        def emit_head_pair(b, g, filler=None, pairs_left=1):
            """Attention for heads (2g, 2g+1) of batch b; the two heads
            occupy PE row groups 0-63 / 64-127 concurrently.
            filler: queue of PSUM-using chunks, popped adaptively at odd jt.
            light: queue of DVE/DMA-only chunks, popped at even jt."""
            heads = (2 * g, 2 * g + 1)
            ps_os = {h: psO.tile([DH + 1, N], F32, tag="psO", name=f"ps_o_{b}_{h}") for h in heads}
            for jt in range(NT):
                tiles = {
                    h: psB.tile([128, N], F32, tag="psB",
                                name=f"ps_s_{b}_{h}_{jt}")
                    for h in heads
                }
                for ih in range(2):
                    for h in heads:
                        q_off = (h % 2) * 64
                        nc.tensor.matmul(
                            tiles[h][:, 512 * ih:512 * ih + 512],
                            qkT[b][q_off:q_off + 64, 4 + g,
                                   128 * jt:128 * jt + 128],
                            qkT[b][q_off:q_off + 64, g,
                                   512 * ih:512 * ih + 512],
                            start=True, stop=True,
                        )
                for h in heads:
                    expT = p_exp.tile([128, N], ATT_DT, tag="exp")
                    nc.scalar.activation(
                        expT, tiles[h], EXP, scale=scale_sb[:, h:h + 1]
                    )
                    nc.gpsimd.affine_select(
                        out=expT[:, 128 * jt:128 * jt + 128],
                        in_=expT[:, 128 * jt:128 * jt + 128],
                        compare_op=mybir.AluOpType.not_equal,
                        fill=0.0, base=0, channel_multiplier=1,
                        pattern=[[-1, 128]],
                    )
                    for ih in range(2):
                        nc.tensor.matmul(
                            ps_os[h][:, 512 * ih:512 * ih + 512],
                            vsb[b][:, jt, VW * h:VW * h + DH + 1],
                            expT[:, 512 * ih:512 * ih + 512],
                            start=(jt == 0), stop=(jt == NT - 1),
                        )
                if filler and jt % 2 == 1:
                    import math as _math
                    npop = max(1, _math.ceil(len(filler) / (pairs_left * 4)))
                    for _ in range(min(npop, len(filler))):
                        filler.pop(0)()
            for h in heads:
                q_off = (h % 2) * 64
                # free the PSUM slot fast: single copy of out^T + sums row
                o_tmp = p_otmp.tile([DH + 1, N], F32, tag="otmp",
                                    name=f"o_tmp_{b}_{h}")
                nc.vector.tensor_copy(o_tmp, ps_os[h])
                sums_sb = p_small.tile([1, N], F32, tag="sums")
                nc.vector.tensor_copy(sums_sb, o_tmp[DH:DH + 1, :])
                recip = p_small.tile([1, N], F32, tag="recip")
                nc.vector.reciprocal_approx_fast(recip, sums_sb)
                scr = p_dram.tile([1, N], F32, tag="scr")
                nc.gpsimd.dma_start(out=scr, in_=recip)
                rb = p_rb.tile([64, N], F32, tag="rb")
                nc.gpsimd.dma_start(
                    out=rb,
                    in_=bass.AP(tensor=scr.tensor, offset=scr.offset,
                                ap=[[0, 64], [1, N]]),
                )
                nc.vector.tensor_mul(
                    osb[b][q_off:q_off + 64, g, :], o_tmp[0:DH, :], rb
                )

        def emit_yproj_r(b, r):
            ps_y = psB.tile([128, 512], F32, tag="psB", name=f"ps_y_{b}_{r}")
            for kt in range(KD):
                nc.tensor.matmul(
                    ps_y,
                    osb[b][:, kt, 128 * r:128 * r + 128],
                    wout_sb[:, kt, :],
                    start=(kt == 0), stop=(kt == KD - 1),
                )
            y_sb = p_y.tile([128, D], F32, tag="y")
            nc.vector.tensor_add(y_sb, ps_y, bout_bc)
            nc.sync.dma_start(
                out=y[b, 128 * r:128 * r + 128, :], in_=y_sb
            )

        # ================= pipelined emission =================
        import functools
        F = functools.partial

        # batch 0 prologue: load + transpose + minimal projection prefix
        x0 = emit_load_x(0)
        xT[0] = p_mid.tile([128, KD, N], F32R, tag="mid", name="xT0")
        qkT[0] = p_qk.tile([128, 8, N], F32R, tag="qk", name="qkT0")
        vsb[0] = p_v.tile([128, NT, HEADS * VW + 64], ATT_DT, tag="v", name="v0")
        osb[0] = p_mid.tile([128, KD, N], F32R, tag="mid", name="o0")
        emit_transposes(0, x0, range(KD))
        emit_ones(0)
        emit_qk_ct(0, 0)       # q heads 0,1
        emit_qk_ct(0, 4)       # k heads 0,1
        for r in range(NT):
            emit_v_r(0, r)

        # batch 1 x-load can start as soon as x0's slot frees
        x1 = emit_load_x(1)
        xT[1] = p_mid.tile([128, KD, N], F32R, tag="mid", name="xT1")

        vsb[1] = p_v.tile([128, NT, HEADS * VW + 64], ATT_DT, tag="v", name="v1")
        osb[1] = p_mid.tile([128, KD, N], F32R, tag="mid", name="o1")

        # C(0) filler queue: remaining B(0) qk chunks + A(1) transposes + v(1)
        q0 = []
        for ct in (1, 5, 2, 6, 3, 7):
            for nh in range(2):
                q0.append(F(emit_qk_half, 0, ct, nh))
        for kd in range(KD):
            for half in range(2):
                q0.append(F(emit_transpose_half, 1, x1, kd, half))
        q0.append(F(emit_ones, 1))
        for r in range(NT):
            q0.append(F(emit_v_r, 1, r))

        for g in range(4):
            emit_head_pair(0, g, filler=q0, pairs_left=4 - g)

        while q0:
            q0.pop(0)()
        # qk pool is single-buffered (fp32r): batch 1 q/k prefix must wait
        # until C(0)'s reads of qkT[0] are done
        qkT[1] = p_qk.tile([128, 8, N], F32R, tag="qk", name="qkT1")
        for ct in (0, 4):
            for nh in range(2):
                emit_qk_half(1, ct, nh)

        # C(1) filler queue: remaining B(1) qk chunks + D(0)
        q1 = []
        for ct in (1, 5, 2, 6, 3, 7):
            for nh in range(2):
                q1.append(F(emit_qk_half, 1, ct, nh))
        for r in range(NT):
            q1.append(F(emit_yproj_r, 0, r))

        for g in range(4):
            emit_head_pair(1, g, filler=q1, pairs_left=4 - g)
        while q1:
            q1.pop(0)()

        # D(1) tail
        for r in range(NT):
            emit_yproj_r(1, r)

    nc.compile()
    return nc


_NC = None


def _get_program():
    global _NC
    if _NC is None:
        _NC = build_program()
    return _NC


def make_in_maps(x, w_qkv, w_out, b_out, scale):
    x = np.ascontiguousarray(np.asarray(x, dtype=np.float32))
    w_qkv = np.ascontiguousarray(np.asarray(w_qkv, dtype=np.float32))
    w_out = np.ascontiguousarray(np.asarray(w_out, dtype=np.float32))
    b_out = np.ascontiguousarray(np.asarray(b_out, dtype=np.float32))
    scale = np.ascontiguousarray(np.asarray(scale, dtype=np.float32))
    return [
        {
            "x": x[c * BPC:(c + 1) * BPC],
            "w_qkv": w_qkv,
            "w_out": w_out,
            "b_out": b_out,
            "scale": scale,
        }
        for c in range(N_CORES)
    ]


def kernel(x, w_qkv, w_out, b_out, scale):
    nc = _get_program()
    in_maps = make_in_maps(x, w_qkv, w_out, b_out, scale)
    res = run_bass_kernel_spmd(nc, in_maps, core_ids=list(range(N_CORES)))
    return np.concatenate([res.results[c]["y"] for c in range(N_CORES)], axis=0)


if __name__ == "__main__":
    rng = np.random.default_rng(0)
    inputs = {
        "x": rng.standard_normal((B, N, D), dtype=np.float32),
        "w_qkv": rng.standard_normal((D, 3 * D), dtype=np.float32) * 0.03,
        "w_out": rng.standard_normal((D, D), dtype=np.float32) * 0.04,
        "b_out": np.zeros(D, dtype=np.float32),
        "scale": np.full(HEADS, DH ** -0.5, dtype=np.float32),
    }
    out = kernel(**inputs)
    print("kernel output", out.shape, out.dtype)
